# revision 35
# baseline (speedup 1.0000x reference)
"""Trainium2 Bass kernel for GQA attention (B=2, S=2048, DIM=2048, H=16, KV=8,
HD=128) with RoPE + causal mask + output projection.

Sharding: 8-way tensor parallelism over heads. Core c computes q heads
{2c, 2c+1} and kv head c end-to-end (QKV projection, RoPE, causal attention),
contributes its transposed attention output to on-device AllGathers, then
computes the output-projection column slice out[:, 256c:256(c+1)] from the
gathered activations. The host only slices inputs and concatenates outputs.

Pipeline (v11):
- per batch: projections (4 x 512-col windows) -> causal attention in 512-col
  query chunks; each chunk finalizes (softmax-normalize + AllGather) right
  after its AV drain so the collective stream starts as early as possible.
- batch 1's attention chunks are interleaved with batch 0's output-projection
  chunks (plus (1,0) under the longest chunk) as PE filler; (1,1..3) trail,
  each consuming its AllGather as it lands.
- v transposes ride inside the next projection window's matmul stream (and
  window 3's inside attention chunk 0), never on a phase boundary.
- PE warm-up matmuls run during the initial weight DMA (HAM clock gate).

Attention inner loop (both heads packed in [128,1024] tiles):
- scores: 2 matmuls into a 2-bank PSUM tile (3-deep rotation so the PE never
  waits on the exp drain); diagonal blocks of chunks >=1 compute only the
  non-fully-masked column suffix.
- ONE exp activation per j-block; the 128-wide diagonal triangle is zeroed
  after exp by a DVE multiply against a shifted 0/1 mask (no mask matmuls).
- denominator: bf16 eacc accumulate on DVE, then a ones(128,128) matmul
  produces the partition-BROADCAST denominator in one shot;
  reciprocal_approx_fast (single custom-DVE op) replaces the 3.3us iterative
  reciprocal previously on the AllGather critical path.
- RoPE: ACT copies the PSUM halves to bf16 SBUF (releasing projection
  accumulators within ~1us, pv/pt single-buffered) and every DVE op runs in
  2x packed-bf16 mode.

DMA queues: activation stream + gathered-chunk loads + output stores ride the
SP queue (gathered loads pinned after the xt stream); weights/collective
bounce ride the Pool queue. Collective triggers block the GpSimd stream until
the previous collective completes, so nothing prompt-critical ever follows a
trigger on that queue. AllGather outputs are addr_space="Shared".

Layout tricks:
- Everything is computed transposed (feature dim on SBUF partitions); the only
  on-device transposes are 16 PE transposes per batch for v.
- RoPE's interleaved (even, odd) pairs are handled by permuting wq/wk columns
  on the host to [evens, odds] per head, making the rotation act on two
  contiguous 64-partition halves. q/k are permuted consistently so q.k dot
  products are unchanged; v / wo stay unpermuted.
- Softmax runs in scoresT layout (keys on partitions): no max subtraction
  (scores are O(5) here), no masking matmuls.
- Matmuls are bf16 (fp32 accumulate); 1/sqrt(HD) is folded into wq.
"""

import sys

if "/opt/trn_rl_repo" not in sys.path:
    sys.path.insert(0, "/opt/trn_rl_repo")

import numpy as np
import ml_dtypes

B, S, DIM = 2, 2048, 2048
H, KV, HD = 16, 8, 128
NC = 8
NS = B * S            # 4096 flattened (b, s) rows
P = 128
MB = DIM // P         # 16 contraction blocks for the projections
BF = ml_dtypes.bfloat16

_cache: dict = {}


def _build(debug=False):
    import concourse.bass as bass
    import concourse.mybir as mybir
    import concourse.tile as tile
    from concourse import bacc
    from concourse.masks import make_identity

    dt = mybir.dt
    f32, bf16, fp16 = dt.float32, dt.bfloat16, dt.float16
    Exp = mybir.ActivationFunctionType.Exp

    nc = bacc.Bacc("TRN2", debug=False, target_bir_lowering=False, num_devices=NC)

    # x^T arrives pre-tiled as [m_block, window, 128, 512] so every
    # projection-stream DMA is one contiguous 128KB block
    xT_h = nc.dram_tensor("xT", (MB, 8, P, 512), bf16, kind="ExternalInput").ap()
    # weights arrive pre-tiled as [mi=128, mb*d] so their DMAs are contiguous
    wq_h = nc.dram_tensor("wq_c", (P, MB * 256), bf16, kind="ExternalInput").ap()
    wk_h = nc.dram_tensor("wk_c", (P, MB * HD), bf16, kind="ExternalInput").ap()
    wv_h = nc.dram_tensor("wv_c", (P, MB * HD), bf16, kind="ExternalInput").ap()
    wo_h = nc.dram_tensor("wo_c", (P, MB * 256), bf16, kind="ExternalInput").ap()
    cos_h = nc.dram_tensor("cosT", (64, NS), bf16, kind="ExternalInput").ap()
    sin_h = nc.dram_tensor("sinT", (64, NS), bf16, kind="ExternalInput").ap()
    tri_h = nc.dram_tensor("tri", (P, 512), bf16, kind="ExternalInput").ap()
    out_h = nc.dram_tensor("outT", (256, NS), f32, kind="ExternalOutput").ap()
    dbg = {}
    if debug:
        for nm, shp in [("qrot_d", (P, 2 * NS)), ("krot_d", (P, NS)),
                        ("vnat_d", (P, NS)), ("oav_d", (P, 2 * NS)),
                        ("ag_d", (NC * 256, NS))]:
            dbg[nm] = nc.dram_tensor(nm, shp, bf16, kind="ExternalOutput").ap()

    with tile.TileContext(nc) as tc:
        with (
            tc.tile_pool(name="const", bufs=1) as const,
            tc.tile_pool(name="persist", bufs=1) as persist,
            tc.tile_pool(name="xs", bufs=8) as xs,
            tc.tile_pool(name="tmp", bufs=3) as tmp,
            tc.tile_pool(name="et", bufs=8) as et,
            tc.tile_pool(name="gp", bufs=24) as gp,
            tc.tile_pool(name="ot", bufs=3) as ot,
            tc.tile_pool(name="dram", bufs=1, space="DRAM") as dram,
        ):
            # ---- constants into SBUF ----
            ident = const.tile([P, P], bf16)
            make_identity(nc, ident[:])
            wq_sb = const.tile([P, MB, 256], bf16)
            nc.gpsimd.dma_start(wq_sb[:], wq_h.rearrange("p (mb d) -> p mb d", mb=MB))
            wk_sb = const.tile([P, MB, HD], bf16)
            nc.gpsimd.dma_start(wk_sb[:], wk_h.rearrange("p (mb d) -> p mb d", mb=MB))
            wv_sb = const.tile([P, MB, HD], bf16)
            nc.gpsimd.dma_start(wv_sb[:], wv_h.rearrange("p (mb d) -> p mb d", mb=MB))
            cos_sb = const.tile([64, NS], bf16)
            nc.gpsimd.dma_start(cos_sb[:], cos_h)
            sin_sb = const.tile([64, NS], bf16)
            nc.gpsimd.dma_start(sin_sb[:], sin_h)
            tri_sb = const.tile([P, 512], bf16)
            nc.gpsimd.dma_start(tri_sb[:], tri_h)
            ones128 = const.tile([P, P], bf16)
            nc.gpsimd.memset(ones128[:], 1.0)
            # wo is not needed until the first output-projection chunk; load
            # it behind everything the projection front section depends on
            wo_sb = const.tile([P, MB, 256], bf16)
            nc.gpsimd.dma_start(wo_sb[:], wo_h.rearrange("p (mb d) -> p mb d", mb=MB))

            # ---- tiny dummy AllGather during the projection front: absorbs
            # the collective bootstrap cost (the first real AllGather
            # otherwise pays an ~11us start delay and runs at half speed)
            agw_i = dram.tile([P, 16], bf16, name="agwi")
            agw_o = dram.tile([NC * P, 16], bf16, addr_space="Shared",
                              name="agwo")
            nc.gpsimd.dma_start(agw_i[:], ones128[:, 0:16])
            nc.gpsimd.collective_compute(
                "AllGather", mybir.AluOpType.bypass,
                replica_groups=[list(range(NC))],
                ins=[agw_i.opt()], outs=[agw_o.opt()],
            )

            # ---- PE warm-up during the initial weight DMA: ~6us of dummy
            # matmuls so the HAM clock gate is open when projections start
            with tc.tile_pool(name="psWu", bufs=1, space="PSUM") as psWu:
                pwu = psWu.tile([P, P], f32)
                for i in range(56):
                    nc.tensor.matmul(pwu[:], ident[:], ident[:],
                                     start=(i == 0), stop=(i == 55))

            # ---- per-batch persistent activations ----
            qrot = [persist.tile([P, 2, S], bf16, name=f"qrot{b}") for b in range(B)]
            krot = [persist.tile([P, S], bf16, name=f"krot{b}") for b in range(B)]
            vTt = [persist.tile([P, S], bf16, name=f"vTt{b}") for b in range(B)]
            vnat = [persist.tile([P, S // P, HD], bf16, name=f"vnat{b}")
                    for b in range(B)]
            ag_in = [[dram.tile([256, 512], bf16, name=f"agi{b}{t}")
                      for t in range(4)] for b in range(B)]
            ag_out = [[dram.tile([NC * 256, 512], bf16, addr_space="Shared",
                                 name=f"ago{b}{t}")
                       for t in range(4)] for b in range(B)]

            last_xt = [None, None]
            from concourse.tile_rust import add_dep_helper

            def rope_unit(src, cos_c, sin_c, out_even, out_odd):
                # ACT first copies the two PSUM halves to bf16 SBUF: this
                # releases the projection accumulator almost immediately (so
                # the PSUM pools can single-buffer) and lets every DVE op run
                # in 2x packed-bf16 mode (327ns vs 658ns for a PSUM-source op)
                ev = tmp.tile([64, 512], bf16, tag="rpe", name="ev")
                od = tmp.tile([64, 512], bf16, tag="rpo", name="od")
                nc.scalar.copy(ev[:], src[0:64, :])
                nc.scalar.copy(od[:], src[64:128, :])
                t1 = tmp.tile([64, 512], bf16, tag="r1", name="r1")
                t2 = tmp.tile([64, 512], bf16, tag="r2", name="r2")
                nc.vector.tensor_mul(t1[:], ev[:], cos_c)
                nc.vector.tensor_mul(t2[:], od[:], sin_c)
                nc.vector.tensor_sub(out_even, t1[:], t2[:])
                t3 = tmp.tile([64, 512], bf16, tag="r1", name="r3")
                t4 = tmp.tile([64, 512], bf16, tag="r2", name="r4")
                nc.vector.tensor_mul(t3[:], ev[:], sin_c)
                nc.vector.tensor_mul(t4[:], od[:], cos_c)
                nc.vector.tensor_add(out_odd, t3[:], t4[:])

            def transpose_v(b, blk, pool):
                # one v-block PE transpose, interleaved into the projection
                # or attention stream so the 16-transpose chain never sits on
                # the critical path between projections and attention
                pt = pool.tile([P, P], bf16, tag=pool.transpose_tag,
                               bufs=pool.transpose_bufs, name="pt")
                nc.tensor.transpose(
                    pt[:], vTt[b][:, blk * P:(blk + 1) * P], ident[:])
                nc.scalar.copy(vnat[b][:, blk, :], pt[:])

            # ---- one fused stream over all 8 projection windows of both
            # batches, one PSUM context: window (b,sp) carries window (b,sp-1)'s
            # v transposes (m=0..3), the previous chunk's attention j-blocks
            # spread over the m-loop (chunk (0,3)'s 16 j-blocks ride inside
            # batch 1's window 0, so neither a sparse standalone chunk nor a
            # pool-transition barrier exists between the batches), and from
            # window 5 on, one gathered output-projection blob per boundary.
            # PSUM: pq0/pq1/pk/pv 1 bank each (released within ~1us by the ACT
            # copies), scores 2x1-bank per-head rotation, pav 2 banks = 8.
            with (
                tc.tile_pool(name="psA", bufs=1, space="PSUM") as psA,
                tc.tile_pool(name="psS", bufs=2, space="PSUM") as psS,
                tc.tile_pool(name="psV", bufs=1, space="PSUM") as psV,
            ):
                psS.transpose_tag = "ps"
                psS.transpose_bufs = 2

                def finalize(cb, pav_f, eacc_f, t_f):
                    """Normalize a finished chunk and fire its AllGather."""
                    rcp = tmp.tile([P, 1024], f32, tag="rcp", name="rcp")
                    for h in range(2):
                        hs = slice(512 * h, 512 * h + 512)
                        dn = psS.tile([P, 512], f32, tag="ps", name="den")
                        nc.tensor.matmul(dn[:], ones128[:], eacc_f[:, hs],
                                         start=True, stop=True)
                        nc.vector.reciprocal_approx_fast(
                            out=rcp[:, hs], in_=dn[:])
                    oavt = tmp.tile([P, 1024], bf16, tag="oav", name="oavt")
                    nc.vector.tensor_mul(oavt[:], pav_f[:], rcp[:])
                    for h in range(2):
                        nc.gpsimd.dma_start(
                            ag_in[cb][t_f][h * P:(h + 1) * P, :],
                            oavt[:, 512 * h:512 * h + 512],
                        )
                    nc.gpsimd.collective_compute(
                        "AllGather",
                        mybir.AluOpType.bypass,
                        replica_groups=[list(range(NC))],
                        ins=[ag_in[cb][t_f].opt()],
                        outs=[ag_out[cb][t_f].opt()],
                    )

                def wo_gen(bb, tt):
                    """Output-projection chunk (bb, tt) as a generator of
                    r-steps (1 gathered row-block load + 2 matmuls each) so it
                    can interleave anywhere the pk/pv PSUM slots are free."""
                    pw = [psA.tile([P, 512], f32, tag=tg, name=f"pw{n}")
                          for n, tg in ((0, "pk"), (1, "pv"))]
                    for r in range(MB):
                        g = gp.tile([P, 512], bf16, tag="g", name="g")
                        nc.sync.dma_start(
                            g[:], ag_out[bb][tt][r * P:(r + 1) * P, :])
                        for n in range(2):
                            nc.tensor.matmul(
                                pw[n][:], wo_sb[:, r, n * 128:(n + 1) * 128],
                                g[:], start=(r == 0), stop=(r == MB - 1),
                            )
                        yield
                    for n in range(2):
                        o = ot.tile([P, 512], f32, tag="o", name="o")
                        nc.scalar.copy(o[:], pw[n][:])
                        nc.sync.dma_start(
                            out_h[n * P:(n + 1) * P,
                                  bb * S + tt * 512: bb * S + (tt + 1) * 512],
                            o[:],
                        )

                def av_pop(cb, pav, entry, nj):
                    ep, ip, jp, lo = entry
                    for h in range(2):
                        nc.tensor.matmul(
                            pav[:, 512 * h + lo:512 * h + 512],
                            vnat[cb][:, jp, :],
                            ep[:, 512 * h + lo:512 * h + 512],
                            start=(ip == 0), stop=(ip == nj - 1),
                        )

                def chunk_gen(cb, t):
                    """Attention chunk t of batch cb in scoresT layout, both
                    heads packed [128,1024]; yields after each j-block so the
                    caller can weave it through the projection stream.
                    Chunk 0 (all-diagonal) runs full-width, descending j so
                    the drain only waits on plain exps; chunks >=1 run the
                    diagonal blocks first ascending, computing only the
                    non-fully-masked column suffix (the first AV issued is
                    then full-width with start=True, which owns the PSUM
                    has_written clear)."""
                    pav = psV.tile([P, 1024], f32, tag="pav", name="pav")
                    eacc = tmp.tile([P, 1024], bf16, tag="eacc", name="eacc")
                    nj = 4 * t + 4
                    if t == 0:
                        order = [3, 2, 1, 0]
                    else:
                        order = [4 * t + r for r in range(4)]
                        order += list(range(4 * t - 1, -1, -1))
                    pipe = []
                    for idx, j in enumerate(order):
                        rel = j - 4 * t
                        partial = rel > 0 and t > 0
                        lo = 128 * rel if partial else 0
                        if t == 3 and idx < 4:
                            # window 3's v transposes; their vnat blocks are
                            # first read by this chunk's diagonal AVs
                            transpose_v(cb, 12 + idx, psS)
                        e = et.tile([P, 1024], bf16, tag="e", name="e")
                        for h in range(2):
                            hs = slice(512 * h + lo, 512 * h + 512)
                            p_ = psS.tile([P, 512], f32, tag="ps", name="ps")
                            nc.tensor.matmul(
                                p_[:, lo:512],
                                krot[cb][:, j * P:(j + 1) * P],
                                qrot[cb][:, h, t * 512 + lo:(t + 1) * 512],
                                start=True, stop=True,
                            )
                            nc.scalar.activation(e[:, hs], p_[:, lo:512], Exp)
                        if rel >= 0:
                            if t == 0:
                                # full-width: zero the whole masked prefix
                                w = 128 * (rel + 1)
                                for h in range(2):
                                    nc.vector.tensor_mul(
                                        e[:, 512 * h:512 * h + w],
                                        e[:, 512 * h:512 * h + w],
                                        tri_sb[:, 384 - 128 * rel:
                                               384 - 128 * rel + w],
                                    )
                            else:
                                # suffix-only: just the 128-wide triangle
                                for h in range(2):
                                    nc.vector.tensor_mul(
                                        e[:, 512 * h + lo:512 * h + lo + 128],
                                        e[:, 512 * h + lo:512 * h + lo + 128],
                                        tri_sb[:, 384:512],
                                    )
                        if idx == 0:
                            nc.vector.tensor_copy(eacc[:], e[:])
                        elif partial:
                            for h in range(2):
                                hs = slice(512 * h + lo, 512 * h + 512)
                                nc.vector.tensor_add(eacc[:, hs],
                                                     eacc[:, hs], e[:, hs])
                        else:
                            nc.vector.tensor_add(eacc[:], eacc[:], e[:])
                        pipe.append((e, idx, j, lo))
                        if len(pipe) > 3:
                            av_pop(cb, pav, pipe.pop(0), nj)
                        yield
                    for entry in pipe:
                        av_pop(cb, pav, entry, nj)
                    finalize(cb, pav, eacc, t)

                chunk = None
                nj_prev = 0
                for bw in range(2 * 4):          # 8 windows across both batches
                    b, sp = divmod(bw, 4)
                    if bw >= 5:
                        # batch 0's gathered chunks are long since AllGathered:
                        # their output projections run as dense PE blobs at the
                        # window boundaries
                        for _ in wo_gen(0, bw - 5):
                            pass
                    if nj_prev >= 16:
                        jpos = set(range(MB))
                    elif nj_prev > 0:
                        step = max(1, 12 // nj_prev)
                        jpos = {4 + k * step for k in range(nj_prev)}
                    else:
                        jpos = set()
                    gw = slice(b * S + sp * 512, b * S + (sp + 1) * 512)
                    lw = slice(sp * 512, (sp + 1) * 512)
                    pq = [psA.tile([P, 512], f32, tag=f"pq{h}", name=f"pq{h}")
                          for h in range(2)]
                    pk = psA.tile([P, 512], f32, tag="pk", name="pk")
                    pv = psA.tile([P, 512], f32, tag="pv", name="pv")
                    for m in range(MB):
                        xt = xs.tile([P, 512], bf16, tag="xt", name="xt")
                        last_xt[b] = nc.sync.dma_start(
                            xt[:], xT_h[m, b * 4 + sp])
                        for acc, lhsT in (
                            (pq[0], wq_sb[:, m, 0:128]),
                            (pq[1], wq_sb[:, m, 128:256]),
                            (pk, wk_sb[:, m, :]),
                            (pv, wv_sb[:, m, :]),
                        ):
                            nc.tensor.matmul(
                                acc[:], lhsT, xt[:],
                                start=(m == 0), stop=(m == MB - 1),
                            )
                        if sp > 0 and m < 4:
                            transpose_v(b, (sp - 1) * 4 + m, psS)
                        if m in jpos:
                            next(chunk, None)
                    if chunk is not None:
                        # leftover j-blocks + AV drain + finalize
                        for _ in chunk:
                            pass
                    cos_c, sin_c = cos_sb[:, gw], sin_sb[:, gw]
                    for h in range(2):
                        rope_unit(pq[h], cos_c, sin_c,
                                  qrot[b][0:64, h, lw], qrot[b][64:128, h, lw])
                    rope_unit(pk, cos_c, sin_c,
                              krot[b][0:64, lw], krot[b][64:128, lw])
                    nc.scalar.copy(vTt[b][:, lw], pv[:])
                    chunk = chunk_gen(b, sp)
                    nj_prev = 4 * sp + 4

                # ---- standalone chunk (1,3), ACT-paced: weave (0,3)'s output
                # projection between its j-blocks as PE filler, then drain the
                # remaining chunks while the last AllGathers land
                filler = wo_gen(0, 3)
                for _ in chunk:
                    next(filler, None)
                for _ in filler:
                    pass
                for t in range(4):
                    for _ in wo_gen(1, t):
                        pass

            if debug:
                for b in range(B):
                    for h in range(2):
                        nc.sync.dma_start(
                            dbg["qrot_d"][:, h * NS + b * S: h * NS + (b + 1) * S],
                            qrot[b][:, h, :])
                    nc.sync.dma_start(dbg["krot_d"][:, b * S:(b + 1) * S], krot[b][:])
                    nc.sync.dma_start(
                        dbg["vnat_d"].rearrange("p (bb d) -> p bb d", bb=NS // P)
                        [:, b * (S // P):(b + 1) * (S // P), :], vnat[b][:])
                    for t in range(4):
                        nc.sync.dma_start(
                            dbg["ag_d"][:, b * S + t * 512: b * S + (t + 1) * 512],
                            ag_out[b][t][:])

    nc.compile()
    return nc


def _prep_inputs(x, freqs_cos, freqs_sin, wq, wk, wv, wo):
    x = np.asarray(x, np.float32).reshape(NS, DIM)
    xT = np.ascontiguousarray(
        x.T.reshape(MB, P, 8, 512).transpose(0, 2, 1, 3)).astype(BF)
    cos = np.asarray(freqs_cos, np.float32)
    sin = np.asarray(freqs_sin, np.float32)
    cosT = np.ascontiguousarray(np.tile(cos, (B, 1)).T).astype(BF)
    sinT = np.ascontiguousarray(np.tile(sin, (B, 1)).T).astype(BF)

    perm = np.r_[np.arange(0, HD, 2), np.arange(1, HD, 2)]
    scale = np.float32(1.0 / np.sqrt(HD))
    wq = np.asarray(wq, np.float32) * scale
    wk = np.asarray(wk, np.float32)
    wv = np.asarray(wv, np.float32)
    wo = np.asarray(wo, np.float32)

    # 0/1 keep-mask for the rel=3 diagonal block; rel=r reads the slice
    # shifted left by 128(3-r).  tri[p, c] = 0 iff c < 384 + p.
    cc, pp = np.meshgrid(np.arange(512), np.arange(P))
    tri = np.ascontiguousarray((cc >= pp + 384).astype(np.float32)).astype(BF)

    def tile_w(w):
        # (2048, d) -> (128, 16*d): row mi holds [mb, d] contiguously
        d = w.shape[1]
        return np.ascontiguousarray(
            w.reshape(MB, P, d).transpose(1, 0, 2).reshape(P, MB * d)).astype(BF)

    in_maps = []
    for c in range(NC):
        wq_c = wq[:, c * 256:(c + 1) * 256]
        wq_cp = np.concatenate([wq_c[:, h * HD + perm] for h in range(2)], axis=1)
        in_maps.append({
            "xT": xT,
            "wq_c": tile_w(wq_cp),
            "wk_c": tile_w(wk[:, c * HD:(c + 1) * HD][:, perm]),
            "wv_c": tile_w(wv[:, c * HD:(c + 1) * HD]),
            "wo_c": tile_w(wo[:, c * 256:(c + 1) * 256]),
            "cosT": cosT,
            "sinT": sinT,
            "tri": tri,
        })
    return in_maps


def _run(inputs, trace=False, **kw):
    from concourse.bass_utils import run_bass_kernel_spmd

    if "nc" not in _cache:
        _cache["nc"] = _build()
    nc = _cache["nc"]
    in_maps = _prep_inputs(**inputs)
    res = run_bass_kernel_spmd(
        nc, in_maps, core_ids=list(range(NC)), trace=trace, **kw
    )
    out = np.empty((NS, DIM), np.float32)
    for c in range(NC):
        out[:, c * 256:(c + 1) * 256] = res.results[c]["outT"].T
    return out.reshape(B, S, DIM), res


def kernel(**inputs) -> np.ndarray:
    out, _ = _run(inputs, trace=False)
    return out


# revision 36
# speedup vs baseline: 1.0197x; 1.0197x over previous
"""Trainium2 Bass kernel for GQA attention (B=2, S=2048, DIM=2048, H=16, KV=8,
HD=128) with RoPE + causal mask + output projection.

Sharding: 8-way tensor parallelism over heads. Core c computes q heads
{2c, 2c+1} and kv head c end-to-end (QKV projection, RoPE, causal attention),
contributes its transposed attention output to on-device AllGathers, then
computes the output-projection column slice out[:, 256c:256(c+1)] from the
gathered activations. The host only slices inputs and concatenates outputs.

Pipeline (v11):
- per batch: projections (4 x 512-col windows) -> causal attention in 512-col
  query chunks; each chunk finalizes (softmax-normalize + AllGather) right
  after its AV drain so the collective stream starts as early as possible.
- batch 1's attention chunks are interleaved with batch 0's output-projection
  chunks (plus (1,0) under the longest chunk) as PE filler; (1,1..3) trail,
  each consuming its AllGather as it lands.
- v transposes ride inside the next projection window's matmul stream (and
  window 3's inside attention chunk 0), never on a phase boundary.
- PE warm-up matmuls run during the initial weight DMA (HAM clock gate).

Attention inner loop (both heads packed in [128,1024] tiles):
- scores: 2 matmuls into a 2-bank PSUM tile (3-deep rotation so the PE never
  waits on the exp drain); diagonal blocks of chunks >=1 compute only the
  non-fully-masked column suffix.
- ONE exp activation per j-block; the 128-wide diagonal triangle is zeroed
  after exp by a DVE multiply against a shifted 0/1 mask (no mask matmuls).
- denominator: bf16 eacc accumulate on DVE, then a ones(128,128) matmul
  produces the partition-BROADCAST denominator in one shot;
  reciprocal_approx_fast (single custom-DVE op) replaces the 3.3us iterative
  reciprocal previously on the AllGather critical path.
- RoPE: ACT copies the PSUM halves to bf16 SBUF (releasing projection
  accumulators within ~1us, pv/pt single-buffered) and every DVE op runs in
  2x packed-bf16 mode.

DMA queues: activation stream + gathered-chunk loads + output stores ride the
SP queue (gathered loads pinned after the xt stream); weights/collective
bounce ride the Pool queue. Collective triggers block the GpSimd stream until
the previous collective completes, so nothing prompt-critical ever follows a
trigger on that queue. AllGather outputs are addr_space="Shared".

Layout tricks:
- Everything is computed transposed (feature dim on SBUF partitions); the only
  on-device transposes are 16 PE transposes per batch for v.
- RoPE's interleaved (even, odd) pairs are handled by permuting wq/wk columns
  on the host to [evens, odds] per head, making the rotation act on two
  contiguous 64-partition halves. q/k are permuted consistently so q.k dot
  products are unchanged; v / wo stay unpermuted.
- Softmax runs in scoresT layout (keys on partitions): no max subtraction
  (scores are O(5) here), no masking matmuls.
- Matmuls are bf16 (fp32 accumulate); 1/sqrt(HD) is folded into wq.
"""

import sys

if "/opt/trn_rl_repo" not in sys.path:
    sys.path.insert(0, "/opt/trn_rl_repo")

import numpy as np
import ml_dtypes

B, S, DIM = 2, 2048, 2048
H, KV, HD = 16, 8, 128
NC = 8
NS = B * S            # 4096 flattened (b, s) rows
P = 128
MB = DIM // P         # 16 contraction blocks for the projections
BF = ml_dtypes.bfloat16

_cache: dict = {}


def _build(debug=False):
    import concourse.bass as bass
    import concourse.mybir as mybir
    import concourse.tile as tile
    from concourse import bacc
    from concourse.masks import make_identity

    dt = mybir.dt
    f32, bf16, fp16 = dt.float32, dt.bfloat16, dt.float16
    Exp = mybir.ActivationFunctionType.Exp

    nc = bacc.Bacc("TRN2", debug=False, target_bir_lowering=False, num_devices=NC)

    # x^T arrives pre-tiled as [m_block, window, 128, 512] so every
    # projection-stream DMA is one contiguous 128KB block
    xT_h = nc.dram_tensor("xT", (MB, 8, P, 512), bf16, kind="ExternalInput").ap()
    # weights arrive pre-tiled as [mi=128, mb*d] so their DMAs are contiguous
    wq_h = nc.dram_tensor("wq_c", (P, MB * 256), bf16, kind="ExternalInput").ap()
    wk_h = nc.dram_tensor("wk_c", (P, MB * HD), bf16, kind="ExternalInput").ap()
    wv_h = nc.dram_tensor("wv_c", (P, MB * HD), bf16, kind="ExternalInput").ap()
    wo_h = nc.dram_tensor("wo_c", (P, MB * 256), bf16, kind="ExternalInput").ap()
    cos_h = nc.dram_tensor("cosT", (64, NS), bf16, kind="ExternalInput").ap()
    sin_h = nc.dram_tensor("sinT", (64, NS), bf16, kind="ExternalInput").ap()
    tri_h = nc.dram_tensor("tri", (P, 512), bf16, kind="ExternalInput").ap()
    out_h = nc.dram_tensor("outT", (256, NS), f32, kind="ExternalOutput").ap()
    dbg = {}
    if debug:
        for nm, shp in [("qrot_d", (P, 2 * NS)), ("krot_d", (P, NS)),
                        ("vnat_d", (P, NS)), ("oav_d", (P, 2 * NS)),
                        ("ag_d", (NC * 256, NS))]:
            dbg[nm] = nc.dram_tensor(nm, shp, bf16, kind="ExternalOutput").ap()

    with tile.TileContext(nc) as tc:
        with (
            tc.tile_pool(name="const", bufs=1) as const,
            tc.tile_pool(name="persist", bufs=1) as persist,
            tc.tile_pool(name="xs", bufs=8) as xs,
            tc.tile_pool(name="tmp", bufs=3) as tmp,
            tc.tile_pool(name="et", bufs=8) as et,
            tc.tile_pool(name="gp", bufs=24) as gp,
            tc.tile_pool(name="ot", bufs=3) as ot,
            tc.tile_pool(name="dram", bufs=1, space="DRAM") as dram,
        ):
            # ---- constants into SBUF ----
            ident = const.tile([P, P], bf16)
            make_identity(nc, ident[:])
            wq_sb = const.tile([P, MB, 256], bf16)
            nc.gpsimd.dma_start(wq_sb[:], wq_h.rearrange("p (mb d) -> p mb d", mb=MB))
            wk_sb = const.tile([P, MB, HD], bf16)
            nc.gpsimd.dma_start(wk_sb[:], wk_h.rearrange("p (mb d) -> p mb d", mb=MB))
            wv_sb = const.tile([P, MB, HD], bf16)
            nc.gpsimd.dma_start(wv_sb[:], wv_h.rearrange("p (mb d) -> p mb d", mb=MB))
            cos_sb = const.tile([64, NS], bf16)
            nc.gpsimd.dma_start(cos_sb[:], cos_h)
            sin_sb = const.tile([64, NS], bf16)
            nc.gpsimd.dma_start(sin_sb[:], sin_h)
            tri_sb = const.tile([P, 512], bf16)
            nc.gpsimd.dma_start(tri_sb[:], tri_h)
            ones128 = const.tile([P, P], bf16)
            nc.gpsimd.memset(ones128[:], 1.0)
            # wo is not needed until the first output-projection chunk; load
            # it behind everything the projection front section depends on
            wo_sb = const.tile([P, MB, 256], bf16)
            nc.gpsimd.dma_start(wo_sb[:], wo_h.rearrange("p (mb d) -> p mb d", mb=MB))

            # ---- tiny dummy AllGather during the projection front: absorbs
            # the collective bootstrap cost (the first real AllGather
            # otherwise pays an ~11us start delay and runs at half speed)
            agw_i = dram.tile([P, 16], bf16, name="agwi")
            agw_o = dram.tile([NC * P, 16], bf16, addr_space="Shared",
                              name="agwo")
            nc.gpsimd.dma_start(agw_i[:], ones128[:, 0:16])
            nc.gpsimd.collective_compute(
                "AllGather", mybir.AluOpType.bypass,
                replica_groups=[list(range(NC))],
                ins=[agw_i.opt()], outs=[agw_o.opt()],
            )

            # ---- PE warm-up during the initial weight DMA: ~6us of dummy
            # matmuls so the HAM clock gate is open when projections start
            with tc.tile_pool(name="psWu", bufs=1, space="PSUM") as psWu:
                pwu = psWu.tile([P, P], f32)
                for i in range(56):
                    nc.tensor.matmul(pwu[:], ident[:], ident[:],
                                     start=(i == 0), stop=(i == 55))

            # ---- per-batch persistent activations ----
            qrot = [persist.tile([P, 2, S], bf16, name=f"qrot{b}") for b in range(B)]
            krot = [persist.tile([P, S], bf16, name=f"krot{b}") for b in range(B)]
            vTt = [persist.tile([P, S], bf16, name=f"vTt{b}") for b in range(B)]
            vnat = [persist.tile([P, S // P, HD], bf16, name=f"vnat{b}")
                    for b in range(B)]
            ag_in = [[dram.tile([256, 512], bf16, name=f"agi{b}{t}")
                      for t in range(4)] for b in range(B)]
            ag_out = [[dram.tile([NC * 256, 512], bf16, addr_space="Shared",
                                 name=f"ago{b}{t}")
                       for t in range(4)] for b in range(B)]

            last_xt = [None, None]
            from concourse.tile_rust import add_dep_helper

            def rope_unit(src, cos_c, sin_c, out_even, out_odd):
                # ACT first copies the two PSUM halves to bf16 SBUF: this
                # releases the projection accumulator almost immediately (so
                # the PSUM pools can single-buffer) and lets every DVE op run
                # in 2x packed-bf16 mode (327ns vs 658ns for a PSUM-source op)
                ev = tmp.tile([64, 512], bf16, tag="rpe", name="ev")
                od = tmp.tile([64, 512], bf16, tag="rpo", name="od")
                nc.scalar.copy(ev[:], src[0:64, :])
                nc.scalar.copy(od[:], src[64:128, :])
                t1 = tmp.tile([64, 512], bf16, tag="r1", name="r1")
                t2 = tmp.tile([64, 512], bf16, tag="r2", name="r2")
                nc.vector.tensor_mul(t1[:], ev[:], cos_c)
                nc.vector.tensor_mul(t2[:], od[:], sin_c)
                nc.vector.tensor_sub(out_even, t1[:], t2[:])
                t3 = tmp.tile([64, 512], bf16, tag="r1", name="r3")
                t4 = tmp.tile([64, 512], bf16, tag="r2", name="r4")
                nc.vector.tensor_mul(t3[:], ev[:], sin_c)
                nc.vector.tensor_mul(t4[:], od[:], cos_c)
                nc.vector.tensor_add(out_odd, t3[:], t4[:])

            def transpose_v(b, blk, pool):
                # one v-block PE transpose, interleaved into the projection
                # or attention stream so the 16-transpose chain never sits on
                # the critical path between projections and attention
                pt = pool.tile([P, P], bf16, tag=pool.transpose_tag,
                               bufs=pool.transpose_bufs, name="pt")
                nc.tensor.transpose(
                    pt[:], vTt[b][:, blk * P:(blk + 1) * P], ident[:])
                nc.scalar.copy(vnat[b][:, blk, :], pt[:])

            # ---- one fused stream over all 8 projection windows of both
            # batches, one PSUM context: window (b,sp) carries window (b,sp-1)'s
            # v transposes (m=0..3), the previous chunk's attention j-blocks
            # spread over the m-loop (chunk (0,3)'s 16 j-blocks ride inside
            # batch 1's window 0, so neither a sparse standalone chunk nor a
            # pool-transition barrier exists between the batches), and from
            # window 5 on, one gathered output-projection blob per boundary.
            # PSUM: pq0/pq1/pk/pv 1 bank each (released within ~1us by the ACT
            # copies), scores 2x1-bank per-head rotation, pav 2 banks = 8.
            with (
                tc.tile_pool(name="psA", bufs=1, space="PSUM") as psA,
                tc.tile_pool(name="psS", bufs=2, space="PSUM") as psS,
                tc.tile_pool(name="psV", bufs=1, space="PSUM") as psV,
            ):
                psS.transpose_tag = "ps"
                psS.transpose_bufs = 2

                def finalize(cb, pav_f, eacc_f, t_f):
                    """Normalize a finished chunk and fire its AllGather."""
                    rcp = tmp.tile([P, 1024], f32, tag="rcp", name="rcp")
                    for h in range(2):
                        hs = slice(512 * h, 512 * h + 512)
                        dn = psS.tile([P, 512], f32, tag="ps", name="den")
                        nc.tensor.matmul(dn[:], ones128[:], eacc_f[:, hs],
                                         start=True, stop=True)
                        nc.vector.reciprocal_approx_fast(
                            out=rcp[:, hs], in_=dn[:])
                    oavt = tmp.tile([P, 1024], bf16, tag="oav", name="oavt")
                    nc.vector.tensor_mul(oavt[:], pav_f[:], rcp[:])
                    for h in range(2):
                        nc.gpsimd.dma_start(
                            ag_in[cb][t_f][h * P:(h + 1) * P, :],
                            oavt[:, 512 * h:512 * h + 512],
                        )
                    nc.gpsimd.collective_compute(
                        "AllGather",
                        mybir.AluOpType.bypass,
                        replica_groups=[list(range(NC))],
                        ins=[ag_in[cb][t_f].opt()],
                        outs=[ag_out[cb][t_f].opt()],
                    )

                def wo_gen(bb, tt):
                    """Output-projection chunk (bb, tt) as a generator of
                    r-steps (1 gathered row-block load + 2 matmuls each) so it
                    can interleave anywhere the pk/pv PSUM slots are free."""
                    pw = [psA.tile([P, 512], f32, tag=tg, name=f"pw{n}")
                          for n, tg in ((0, "pk"), (1, "pv"))]
                    for r in range(MB):
                        g = gp.tile([P, 512], bf16, tag="g", name="g")
                        nc.sync.dma_start(
                            g[:], ag_out[bb][tt][r * P:(r + 1) * P, :])
                        for n in range(2):
                            nc.tensor.matmul(
                                pw[n][:], wo_sb[:, r, n * 128:(n + 1) * 128],
                                g[:], start=(r == 0), stop=(r == MB - 1),
                            )
                        yield
                    for n in range(2):
                        o = ot.tile([P, 512], f32, tag="o", name="o")
                        nc.scalar.copy(o[:], pw[n][:])
                        nc.sync.dma_start(
                            out_h[n * P:(n + 1) * P,
                                  bb * S + tt * 512: bb * S + (tt + 1) * 512],
                            o[:],
                        )

                def av_pop(cb, pav, entry, nj):
                    ep, ip, jp, lo = entry
                    for h in range(2):
                        nc.tensor.matmul(
                            pav[:, 512 * h + lo:512 * h + 512],
                            vnat[cb][:, jp, :],
                            ep[:, 512 * h + lo:512 * h + 512],
                            start=(ip == 0), stop=(ip == nj - 1),
                        )

                def chunk_gen(cb, t):
                    """Attention chunk t of batch cb in scoresT layout, both
                    heads packed [128,1024]; yields after each j-block so the
                    caller can weave it through the projection stream.
                    Chunk 0 (all-diagonal) runs full-width, descending j so
                    the drain only waits on plain exps; chunks >=1 run the
                    diagonal blocks first ascending, computing only the
                    non-fully-masked column suffix (the first AV issued is
                    then full-width with start=True, which owns the PSUM
                    has_written clear)."""
                    pav = psV.tile([P, 1024], f32, tag="pav", name="pav")
                    eacc = tmp.tile([P, 1024], bf16, tag="eacc", name="eacc")
                    nj = 4 * t + 4
                    if t == 0:
                        order = [3, 2, 1, 0]
                    else:
                        order = [4 * t + r for r in range(4)]
                        order += list(range(4 * t - 1, -1, -1))
                    pipe = []
                    for idx, j in enumerate(order):
                        rel = j - 4 * t
                        partial = rel > 0 and t > 0
                        lo = 128 * rel if partial else 0
                        if t == 3 and idx < 4:
                            # window 3's v transposes; their vnat blocks are
                            # first read by this chunk's diagonal AVs
                            transpose_v(cb, 12 + idx, psS)
                        e = et.tile([P, 1024], bf16, tag="e", name="e")
                        for h in range(2):
                            hs = slice(512 * h + lo, 512 * h + 512)
                            p_ = psS.tile([P, 512], f32, tag="ps", name="ps")
                            nc.tensor.matmul(
                                p_[:, lo:512],
                                krot[cb][:, j * P:(j + 1) * P],
                                qrot[cb][:, h, t * 512 + lo:(t + 1) * 512],
                                start=True, stop=True,
                            )
                            nc.scalar.activation(e[:, hs], p_[:, lo:512], Exp)
                        if rel >= 0:
                            if t == 0:
                                # full-width: zero the whole masked prefix
                                w = 128 * (rel + 1)
                                for h in range(2):
                                    nc.vector.tensor_mul(
                                        e[:, 512 * h:512 * h + w],
                                        e[:, 512 * h:512 * h + w],
                                        tri_sb[:, 384 - 128 * rel:
                                               384 - 128 * rel + w],
                                    )
                            else:
                                # suffix-only: just the 128-wide triangle
                                for h in range(2):
                                    nc.vector.tensor_mul(
                                        e[:, 512 * h + lo:512 * h + lo + 128],
                                        e[:, 512 * h + lo:512 * h + lo + 128],
                                        tri_sb[:, 384:512],
                                    )
                        if idx == 0:
                            nc.vector.tensor_copy(eacc[:], e[:])
                        elif partial:
                            for h in range(2):
                                hs = slice(512 * h + lo, 512 * h + 512)
                                nc.vector.tensor_add(eacc[:, hs],
                                                     eacc[:, hs], e[:, hs])
                        else:
                            nc.vector.tensor_add(eacc[:], eacc[:], e[:])
                        pipe.append((e, idx, j, lo))
                        if len(pipe) > 3:
                            av_pop(cb, pav, pipe.pop(0), nj)
                        yield
                    for entry in pipe:
                        av_pop(cb, pav, entry, nj)
                    finalize(cb, pav, eacc, t)

                chunk = None
                nj_prev = 0
                for bw in range(2 * 4):          # 8 windows across both batches
                    b, sp = divmod(bw, 4)
                    if bw >= 5:
                        # batch 0's gathered chunks are long since AllGathered:
                        # their output projections run as dense PE blobs at the
                        # window boundaries
                        for _ in wo_gen(0, bw - 5):
                            pass
                    if nj_prev >= 16:
                        jpos = {m: 1 for m in range(3, 13)}
                        jpos.update({13: 2, 14: 2, 15: 2})
                    elif nj_prev > 0:
                        step = max(1, 12 // nj_prev)
                        jpos = {4 + k * step: 1 for k in range(nj_prev)}
                    else:
                        jpos = {}
                    gw = slice(b * S + sp * 512, b * S + (sp + 1) * 512)
                    lw = slice(sp * 512, (sp + 1) * 512)
                    pq = [psA.tile([P, 512], f32, tag=f"pq{h}", name=f"pq{h}")
                          for h in range(2)]
                    pk = psA.tile([P, 512], f32, tag="pk", name="pk")
                    pv = psA.tile([P, 512], f32, tag="pv", name="pv")
                    for m in range(MB):
                        xt = xs.tile([P, 512], bf16, tag="xt", name="xt")
                        last_xt[b] = nc.sync.dma_start(
                            xt[:], xT_h[m, b * 4 + sp])
                        for acc, lhsT in (
                            (pq[0], wq_sb[:, m, 0:128]),
                            (pq[1], wq_sb[:, m, 128:256]),
                            (pk, wk_sb[:, m, :]),
                            (pv, wv_sb[:, m, :]),
                        ):
                            nc.tensor.matmul(
                                acc[:], lhsT, xt[:],
                                start=(m == 0), stop=(m == MB - 1),
                            )
                        if sp > 0 and m < 4:
                            transpose_v(b, (sp - 1) * 4 + m, psS)
                        for _ in range(jpos.get(m, 0)):
                            next(chunk, None)
                    cos_c, sin_c = cos_sb[:, gw], sin_sb[:, gw]
                    for h in range(2):
                        rope_unit(pq[h], cos_c, sin_c,
                                  qrot[b][0:64, h, lw], qrot[b][64:128, h, lw])
                    rope_unit(pk, cos_c, sin_c,
                              krot[b][0:64, lw], krot[b][64:128, lw])
                    nc.scalar.copy(vTt[b][:, lw], pv[:])
                    if chunk is not None:
                        # leftover j-blocks + AV drain + finalize
                        for _ in chunk:
                            pass
                    chunk = chunk_gen(b, sp)
                    nj_prev = 4 * sp + 4

                # ---- standalone chunk (1,3), ACT-paced: weave (0,3)'s output
                # projection between its j-blocks as PE filler, then drain the
                # remaining chunks while the last AllGathers land
                filler = wo_gen(0, 3)
                for _ in chunk:
                    next(filler, None)
                for _ in filler:
                    pass
                for t in range(4):
                    for _ in wo_gen(1, t):
                        pass

            if debug:
                for b in range(B):
                    for h in range(2):
                        nc.sync.dma_start(
                            dbg["qrot_d"][:, h * NS + b * S: h * NS + (b + 1) * S],
                            qrot[b][:, h, :])
                    nc.sync.dma_start(dbg["krot_d"][:, b * S:(b + 1) * S], krot[b][:])
                    nc.sync.dma_start(
                        dbg["vnat_d"].rearrange("p (bb d) -> p bb d", bb=NS // P)
                        [:, b * (S // P):(b + 1) * (S // P), :], vnat[b][:])
                    for t in range(4):
                        nc.sync.dma_start(
                            dbg["ag_d"][:, b * S + t * 512: b * S + (t + 1) * 512],
                            ag_out[b][t][:])

    nc.compile()
    return nc


def _prep_inputs(x, freqs_cos, freqs_sin, wq, wk, wv, wo):
    x = np.asarray(x, np.float32).reshape(NS, DIM)
    xT = np.ascontiguousarray(
        x.T.reshape(MB, P, 8, 512).transpose(0, 2, 1, 3)).astype(BF)
    cos = np.asarray(freqs_cos, np.float32)
    sin = np.asarray(freqs_sin, np.float32)
    cosT = np.ascontiguousarray(np.tile(cos, (B, 1)).T).astype(BF)
    sinT = np.ascontiguousarray(np.tile(sin, (B, 1)).T).astype(BF)

    perm = np.r_[np.arange(0, HD, 2), np.arange(1, HD, 2)]
    scale = np.float32(1.0 / np.sqrt(HD))
    wq = np.asarray(wq, np.float32) * scale
    wk = np.asarray(wk, np.float32)
    wv = np.asarray(wv, np.float32)
    wo = np.asarray(wo, np.float32)

    # 0/1 keep-mask for the rel=3 diagonal block; rel=r reads the slice
    # shifted left by 128(3-r).  tri[p, c] = 0 iff c < 384 + p.
    cc, pp = np.meshgrid(np.arange(512), np.arange(P))
    tri = np.ascontiguousarray((cc >= pp + 384).astype(np.float32)).astype(BF)

    def tile_w(w):
        # (2048, d) -> (128, 16*d): row mi holds [mb, d] contiguously
        d = w.shape[1]
        return np.ascontiguousarray(
            w.reshape(MB, P, d).transpose(1, 0, 2).reshape(P, MB * d)).astype(BF)

    in_maps = []
    for c in range(NC):
        wq_c = wq[:, c * 256:(c + 1) * 256]
        wq_cp = np.concatenate([wq_c[:, h * HD + perm] for h in range(2)], axis=1)
        in_maps.append({
            "xT": xT,
            "wq_c": tile_w(wq_cp),
            "wk_c": tile_w(wk[:, c * HD:(c + 1) * HD][:, perm]),
            "wv_c": tile_w(wv[:, c * HD:(c + 1) * HD]),
            "wo_c": tile_w(wo[:, c * 256:(c + 1) * 256]),
            "cosT": cosT,
            "sinT": sinT,
            "tri": tri,
        })
    return in_maps


def _run(inputs, trace=False, **kw):
    from concourse.bass_utils import run_bass_kernel_spmd

    if "nc" not in _cache:
        _cache["nc"] = _build()
    nc = _cache["nc"]
    in_maps = _prep_inputs(**inputs)
    res = run_bass_kernel_spmd(
        nc, in_maps, core_ids=list(range(NC)), trace=trace, **kw
    )
    out = np.empty((NS, DIM), np.float32)
    for c in range(NC):
        out[:, c * 256:(c + 1) * 256] = res.results[c]["outT"].T
    return out.reshape(B, S, DIM), res


def kernel(**inputs) -> np.ndarray:
    out, _ = _run(inputs, trace=False)
    return out


# revision 37
# speedup vs baseline: 1.0218x; 1.0020x over previous
"""Trainium2 Bass kernel for GQA attention (B=2, S=2048, DIM=2048, H=16, KV=8,
HD=128) with RoPE + causal mask + output projection.

Sharding: 8-way tensor parallelism over heads. Core c computes q heads
{2c, 2c+1} and kv head c end-to-end (QKV projection, RoPE, causal attention),
contributes its transposed attention output to on-device AllGathers, then
computes the output-projection column slice out[:, 256c:256(c+1)] from the
gathered activations. The host only slices inputs and concatenates outputs.

Pipeline (v11):
- per batch: projections (4 x 512-col windows) -> causal attention in 512-col
  query chunks; each chunk finalizes (softmax-normalize + AllGather) right
  after its AV drain so the collective stream starts as early as possible.
- batch 1's attention chunks are interleaved with batch 0's output-projection
  chunks (plus (1,0) under the longest chunk) as PE filler; (1,1..3) trail,
  each consuming its AllGather as it lands.
- v transposes ride inside the next projection window's matmul stream (and
  window 3's inside attention chunk 0), never on a phase boundary.
- PE warm-up matmuls run during the initial weight DMA (HAM clock gate).

Attention inner loop (both heads packed in [128,1024] tiles):
- scores: 2 matmuls into a 2-bank PSUM tile (3-deep rotation so the PE never
  waits on the exp drain); diagonal blocks of chunks >=1 compute only the
  non-fully-masked column suffix.
- ONE exp activation per j-block; the 128-wide diagonal triangle is zeroed
  after exp by a DVE multiply against a shifted 0/1 mask (no mask matmuls).
- denominator: bf16 eacc accumulate on DVE, then a ones(128,128) matmul
  produces the partition-BROADCAST denominator in one shot;
  reciprocal_approx_fast (single custom-DVE op) replaces the 3.3us iterative
  reciprocal previously on the AllGather critical path.
- RoPE: ACT copies the PSUM halves to bf16 SBUF (releasing projection
  accumulators within ~1us, pv/pt single-buffered) and every DVE op runs in
  2x packed-bf16 mode.

DMA queues: activation stream + gathered-chunk loads + output stores ride the
SP queue (gathered loads pinned after the xt stream); weights/collective
bounce ride the Pool queue. Collective triggers block the GpSimd stream until
the previous collective completes, so nothing prompt-critical ever follows a
trigger on that queue. AllGather outputs are addr_space="Shared".

Layout tricks:
- Everything is computed transposed (feature dim on SBUF partitions); the only
  on-device transposes are 16 PE transposes per batch for v.
- RoPE's interleaved (even, odd) pairs are handled by permuting wq/wk columns
  on the host to [evens, odds] per head, making the rotation act on two
  contiguous 64-partition halves. q/k are permuted consistently so q.k dot
  products are unchanged; v / wo stay unpermuted.
- Softmax runs in scoresT layout (keys on partitions): no max subtraction
  (scores are O(5) here), no masking matmuls.
- Matmuls are bf16 (fp32 accumulate); 1/sqrt(HD) is folded into wq.
"""

import sys

if "/opt/trn_rl_repo" not in sys.path:
    sys.path.insert(0, "/opt/trn_rl_repo")

import numpy as np
import ml_dtypes

B, S, DIM = 2, 2048, 2048
H, KV, HD = 16, 8, 128
NC = 8
NS = B * S            # 4096 flattened (b, s) rows
P = 128
MB = DIM // P         # 16 contraction blocks for the projections
BF = ml_dtypes.bfloat16

_cache: dict = {}


def _build(debug=False):
    import concourse.bass as bass
    import concourse.mybir as mybir
    import concourse.tile as tile
    from concourse import bacc
    from concourse.masks import make_identity

    dt = mybir.dt
    f32, bf16, fp16 = dt.float32, dt.bfloat16, dt.float16
    Exp = mybir.ActivationFunctionType.Exp

    nc = bacc.Bacc("TRN2", debug=False, target_bir_lowering=False, num_devices=NC)

    # x^T arrives pre-tiled as [m_block, window, 128, 512] so every
    # projection-stream DMA is one contiguous 128KB block
    xT_h = nc.dram_tensor("xT", (MB, 8, P, 512), bf16, kind="ExternalInput").ap()
    # weights arrive pre-tiled as [mi=128, mb*d] so their DMAs are contiguous
    wq_h = nc.dram_tensor("wq_c", (P, MB * 256), bf16, kind="ExternalInput").ap()
    wk_h = nc.dram_tensor("wk_c", (P, MB * HD), bf16, kind="ExternalInput").ap()
    wv_h = nc.dram_tensor("wv_c", (P, MB * HD), bf16, kind="ExternalInput").ap()
    wo_h = nc.dram_tensor("wo_c", (P, MB * 256), bf16, kind="ExternalInput").ap()
    cos_h = nc.dram_tensor("cosT", (64, NS), bf16, kind="ExternalInput").ap()
    sin_h = nc.dram_tensor("sinT", (64, NS), bf16, kind="ExternalInput").ap()
    tri_h = nc.dram_tensor("tri", (P, 512), bf16, kind="ExternalInput").ap()
    out_h = nc.dram_tensor("outT", (256, NS), f32, kind="ExternalOutput").ap()
    dbg = {}
    if debug:
        for nm, shp in [("qrot_d", (P, 2 * NS)), ("krot_d", (P, NS)),
                        ("vnat_d", (P, NS)), ("oav_d", (P, 2 * NS)),
                        ("ag_d", (NC * 256, NS))]:
            dbg[nm] = nc.dram_tensor(nm, shp, bf16, kind="ExternalOutput").ap()

    with tile.TileContext(nc) as tc:
        with (
            tc.tile_pool(name="const", bufs=1) as const,
            tc.tile_pool(name="persist", bufs=1) as persist,
            tc.tile_pool(name="xs", bufs=8) as xs,
            tc.tile_pool(name="tmp", bufs=3) as tmp,
            tc.tile_pool(name="et", bufs=8) as et,
            tc.tile_pool(name="gp", bufs=24) as gp,
            tc.tile_pool(name="ot", bufs=3) as ot,
            tc.tile_pool(name="dram", bufs=1, space="DRAM") as dram,
        ):
            # ---- constants into SBUF ----
            ident = const.tile([P, P], bf16)
            make_identity(nc, ident[:])
            wq_sb = const.tile([P, MB, 256], bf16)
            nc.gpsimd.dma_start(wq_sb[:], wq_h.rearrange("p (mb d) -> p mb d", mb=MB))
            wk_sb = const.tile([P, MB, HD], bf16)
            nc.gpsimd.dma_start(wk_sb[:], wk_h.rearrange("p (mb d) -> p mb d", mb=MB))
            wv_sb = const.tile([P, MB, HD], bf16)
            nc.gpsimd.dma_start(wv_sb[:], wv_h.rearrange("p (mb d) -> p mb d", mb=MB))
            cos_sb = const.tile([64, NS], bf16)
            nc.gpsimd.dma_start(cos_sb[:], cos_h)
            sin_sb = const.tile([64, NS], bf16)
            nc.gpsimd.dma_start(sin_sb[:], sin_h)
            tri_sb = const.tile([P, 512], bf16)
            nc.gpsimd.dma_start(tri_sb[:], tri_h)
            ones128 = const.tile([P, P], bf16)
            nc.gpsimd.memset(ones128[:], 1.0)
            # wo is not needed until the first output-projection chunk; load
            # it behind everything the projection front section depends on
            wo_sb = const.tile([P, MB, 256], bf16)
            nc.gpsimd.dma_start(wo_sb[:], wo_h.rearrange("p (mb d) -> p mb d", mb=MB))

            # ---- tiny dummy AllGather during the projection front: absorbs
            # the collective bootstrap cost (the first real AllGather
            # otherwise pays an ~11us start delay and runs at half speed)
            agw_i = dram.tile([P, 16], bf16, name="agwi")
            agw_o = dram.tile([NC * P, 16], bf16, addr_space="Shared",
                              name="agwo")
            nc.gpsimd.dma_start(agw_i[:], ones128[:, 0:16])
            nc.gpsimd.collective_compute(
                "AllGather", mybir.AluOpType.bypass,
                replica_groups=[list(range(NC))],
                ins=[agw_i.opt()], outs=[agw_o.opt()],
            )

            # ---- PE warm-up during the initial weight DMA: ~6us of dummy
            # matmuls so the HAM clock gate is open when projections start
            with tc.tile_pool(name="psWu", bufs=1, space="PSUM") as psWu:
                pwu = psWu.tile([P, P], f32)
                for i in range(56):
                    nc.tensor.matmul(pwu[:], ident[:], ident[:],
                                     start=(i == 0), stop=(i == 55))

            # ---- per-batch persistent activations ----
            qrot = [persist.tile([P, 2, S], bf16, name=f"qrot{b}") for b in range(B)]
            krot = [persist.tile([P, S], bf16, name=f"krot{b}") for b in range(B)]
            vTt = [persist.tile([P, S], bf16, name=f"vTt{b}") for b in range(B)]
            vnat = [persist.tile([P, S // P, HD], bf16, name=f"vnat{b}")
                    for b in range(B)]
            ag_in = [[dram.tile([256, 512], bf16, name=f"agi{b}{t}")
                      for t in range(4)] for b in range(B)]
            ag_out = [[dram.tile([NC * 256, 512], bf16, addr_space="Shared",
                                 name=f"ago{b}{t}")
                       for t in range(4)] for b in range(B)]

            last_xt = [None, None]
            from concourse.tile_rust import add_dep_helper

            def rope_unit(src, cos_c, sin_c, out_even, out_odd):
                # ACT first copies the two PSUM halves to bf16 SBUF: this
                # releases the projection accumulator almost immediately (so
                # the PSUM pools can single-buffer) and lets every DVE op run
                # in 2x packed-bf16 mode (327ns vs 658ns for a PSUM-source op)
                ev = tmp.tile([64, 512], bf16, tag="rpe", name="ev")
                od = tmp.tile([64, 512], bf16, tag="rpo", name="od")
                nc.scalar.copy(ev[:], src[0:64, :])
                nc.scalar.copy(od[:], src[64:128, :])
                t1 = tmp.tile([64, 512], bf16, tag="r1", name="r1")
                t2 = tmp.tile([64, 512], bf16, tag="r2", name="r2")
                nc.vector.tensor_mul(t1[:], ev[:], cos_c)
                nc.vector.tensor_mul(t2[:], od[:], sin_c)
                nc.vector.tensor_sub(out_even, t1[:], t2[:])
                t3 = tmp.tile([64, 512], bf16, tag="r1", name="r3")
                t4 = tmp.tile([64, 512], bf16, tag="r2", name="r4")
                nc.vector.tensor_mul(t3[:], ev[:], sin_c)
                nc.vector.tensor_mul(t4[:], od[:], cos_c)
                nc.vector.tensor_add(out_odd, t3[:], t4[:])

            def transpose_v(b, blk, pool):
                # one v-block PE transpose, interleaved into the projection
                # or attention stream so the 16-transpose chain never sits on
                # the critical path between projections and attention
                pt = pool.tile([P, P], bf16, tag=pool.transpose_tag,
                               bufs=pool.transpose_bufs, name="pt")
                nc.tensor.transpose(
                    pt[:], vTt[b][:, blk * P:(blk + 1) * P], ident[:])
                nc.scalar.copy(vnat[b][:, blk, :], pt[:])

            for b in range(B):
                # ---- fused projection + attention + output projection.
                # Window sp's matmul stream carries: window sp-1's v
                # transposes (m=0..3), attention chunk sp-1's j-blocks
                # (spread over m>=4), and the xt/proj stream itself.  Chunk 3
                # runs standalone after window 3 with output-projection
                # r-steps as fine-grained PE filler between its ACT-paced
                # j-blocks.  PSUM: pq0/pq1/pk/pv 1 bank each (released within
                # ~1us by the ACT copies), scores 2x1-bank per-head rotation,
                # pav 2 banks -- exactly 8.
                with (
                    tc.tile_pool(name=f"psA{b}", bufs=1, space="PSUM") as psA,
                    tc.tile_pool(name=f"psS{b}", bufs=2, space="PSUM") as psS,
                    tc.tile_pool(name=f"psV{b}", bufs=1, space="PSUM") as psV,
                ):
                    psS.transpose_tag = "ps"
                    psS.transpose_bufs = 2

                    def finalize(pav_f, eacc_f, t_f):
                        """Normalize a finished chunk and fire its AllGather.
                        The denominator matmuls go straight onto the PE
                        stream; the DVE reciprocal+scale overlap whatever
                        runs next."""
                        rcp = tmp.tile([P, 1024], f32, tag="rcp", name="rcp")
                        for h in range(2):
                            hs = slice(512 * h, 512 * h + 512)
                            dn = psS.tile([P, 512], f32, tag="ps", name="den")
                            nc.tensor.matmul(dn[:], ones128[:], eacc_f[:, hs],
                                             start=True, stop=True)
                            nc.vector.reciprocal_approx_fast(
                                out=rcp[:, hs], in_=dn[:])
                        oavt = tmp.tile([P, 1024], bf16, tag="oav", name="oavt")
                        nc.vector.tensor_mul(oavt[:], pav_f[:], rcp[:])
                        for h in range(2):
                            nc.gpsimd.dma_start(
                                ag_in[b][t_f][h * P:(h + 1) * P, :],
                                oavt[:, 512 * h:512 * h + 512],
                            )
                        nc.gpsimd.collective_compute(
                            "AllGather",
                            mybir.AluOpType.bypass,
                            replica_groups=[list(range(NC))],
                            ins=[ag_in[b][t_f].opt()],
                            outs=[ag_out[b][t_f].opt()],
                        )

                    def wo_gen(bb, tt):
                        """Output-projection chunk (bb, tt) as a generator of
                        r-steps (1 gathered row-block load + 2 matmuls each)
                        so it can interleave anywhere the pk/pv PSUM slots
                        are free."""
                        pw = [psA.tile([P, 512], f32, tag=tg, name=f"pw{n}")
                              for n, tg in ((0, "pk"), (1, "pv"))]
                        for r in range(MB):
                            g = gp.tile([P, 512], bf16, tag="g", name="g")
                            nc.sync.dma_start(
                                g[:], ag_out[bb][tt][r * P:(r + 1) * P, :])
                            for n in range(2):
                                nc.tensor.matmul(
                                    pw[n][:], wo_sb[:, r, n * 128:(n + 1) * 128],
                                    g[:], start=(r == 0), stop=(r == MB - 1),
                                )
                            yield
                        for n in range(2):
                            o = ot.tile([P, 512], f32, tag="o", name="o")
                            nc.scalar.copy(o[:], pw[n][:])
                            nc.sync.dma_start(
                                out_h[n * P:(n + 1) * P,
                                      bb * S + tt * 512: bb * S + (tt + 1) * 512],
                                o[:],
                            )

                    def av_pop(pav, entry, nj):
                        ep, ip, jp, lo = entry
                        for h in range(2):
                            nc.tensor.matmul(
                                pav[:, 512 * h + lo:512 * h + 512],
                                vnat[b][:, jp, :],
                                ep[:, 512 * h + lo:512 * h + 512],
                                start=(ip == 0), stop=(ip == nj - 1),
                            )

                    def chunk_gen(t):
                        """Attention chunk t in scoresT layout, both heads
                        packed [128,1024]; yields after each j-block so the
                        caller can weave it through the projection stream.
                        Chunk 0 (all-diagonal) runs full-width, descending j
                        so the drain only waits on plain exps; chunks >=1 run
                        the diagonal blocks first ascending, computing only
                        the non-fully-masked column suffix (the first AV
                        issued is then full-width with start=True, which owns
                        the PSUM has_written clear)."""
                        pav = psV.tile([P, 1024], f32, tag="pav", name="pav")
                        eacc = tmp.tile([P, 1024], bf16, tag="eacc", name="eacc")
                        nj = 4 * t + 4
                        if t == 0:
                            order = [3, 2, 1, 0]
                        else:
                            order = [4 * t + r for r in range(4)]
                            order += list(range(4 * t - 1, -1, -1))
                        pipe = []
                        for idx, j in enumerate(order):
                            rel = j - 4 * t
                            partial = rel > 0 and t > 0
                            lo = 128 * rel if partial else 0
                            if t == 3 and idx < 4:
                                # window 3's v transposes; their vnat blocks
                                # are first read by this chunk's diagonal AVs
                                # (popped 3 iterations later)
                                transpose_v(b, 12 + idx, psS)
                            e = et.tile([P, 1024], bf16, tag="e", name="e")
                            for h in range(2):
                                hs = slice(512 * h + lo, 512 * h + 512)
                                p_ = psS.tile([P, 512], f32, tag="ps", name="ps")
                                nc.tensor.matmul(
                                    p_[:, lo:512],
                                    krot[b][:, j * P:(j + 1) * P],
                                    qrot[b][:, h, t * 512 + lo:(t + 1) * 512],
                                    start=True, stop=True,
                                )
                                nc.scalar.activation(e[:, hs], p_[:, lo:512], Exp)
                            if rel >= 0:
                                if t == 0:
                                    # full-width: zero the whole masked prefix
                                    w = 128 * (rel + 1)
                                    for h in range(2):
                                        nc.vector.tensor_mul(
                                            e[:, 512 * h:512 * h + w],
                                            e[:, 512 * h:512 * h + w],
                                            tri_sb[:, 384 - 128 * rel:
                                                   384 - 128 * rel + w],
                                        )
                                else:
                                    # suffix-only: just the 128-wide triangle
                                    for h in range(2):
                                        nc.vector.tensor_mul(
                                            e[:, 512 * h + lo:512 * h + lo + 128],
                                            e[:, 512 * h + lo:512 * h + lo + 128],
                                            tri_sb[:, 384:512],
                                        )
                            if idx == 0:
                                nc.vector.tensor_copy(eacc[:], e[:])
                            elif partial:
                                for h in range(2):
                                    hs = slice(512 * h + lo, 512 * h + 512)
                                    nc.vector.tensor_add(eacc[:, hs],
                                                         eacc[:, hs], e[:, hs])
                            else:
                                nc.vector.tensor_add(eacc[:], eacc[:], e[:])
                            pipe.append((e, idx, j, lo))
                            if len(pipe) > 3:
                                av_pop(pav, pipe.pop(0), nj)
                            yield
                        for entry in pipe:
                            av_pop(pav, entry, nj)
                        finalize(pav, eacc, t)

                    chunk = None
                    for sp in range(4):          # 512-col windows within batch
                        if b == 1 and sp in (1, 2):
                            # batch 0's gathered chunks are long since
                            # AllGathered: their output projections run as
                            # dense PE blobs at the window boundary
                            for _ in wo_gen(0, sp - 1):
                                pass
                        nj_prev = 4 * sp
                        jpos = {}
                        if sp > 0:
                            step = max(1, 12 // nj_prev)
                            jpos = {4 + k * step for k in range(nj_prev)}
                        gw = slice(b * S + sp * 512, b * S + (sp + 1) * 512)
                        lw = slice(sp * 512, (sp + 1) * 512)
                        pq = [psA.tile([P, 512], f32, tag=f"pq{h}", name=f"pq{h}")
                              for h in range(2)]
                        pk = psA.tile([P, 512], f32, tag="pk", name="pk")
                        pv = psA.tile([P, 512], f32, tag="pv", name="pv")
                        for m in range(MB):
                            xt = xs.tile([P, 512], bf16, tag="xt", name="xt")
                            last_xt[b] = nc.sync.dma_start(
                                xt[:], xT_h[m, b * 4 + sp])
                            for acc, lhsT in (
                                (pq[0], wq_sb[:, m, 0:128]),
                                (pq[1], wq_sb[:, m, 128:256]),
                                (pk, wk_sb[:, m, :]),
                                (pv, wv_sb[:, m, :]),
                            ):
                                nc.tensor.matmul(
                                    acc[:], lhsT, xt[:],
                                    start=(m == 0), stop=(m == MB - 1),
                                )
                            if sp > 0 and m < 4:
                                transpose_v(b, (sp - 1) * 4 + m, psS)
                            if m in jpos:
                                next(chunk, None)
                        if chunk is not None:
                            # leftover j-blocks + AV drain + finalize of
                            # chunk sp-1 before the RoPE of this window
                            for _ in chunk:
                                pass
                        cos_c, sin_c = cos_sb[:, gw], sin_sb[:, gw]
                        for h in range(2):
                            rope_unit(pq[h], cos_c, sin_c,
                                      qrot[b][0:64, h, lw], qrot[b][64:128, h, lw])
                        rope_unit(pk, cos_c, sin_c,
                                  krot[b][0:64, lw], krot[b][64:128, lw])
                        nc.scalar.copy(vTt[b][:, lw], pv[:])
                        chunk = chunk_gen(sp)

                    # ---- standalone chunk 3, ACT-paced: weave output
                    # projection r-steps between its j-blocks as PE filler
                    if b == 0:
                        # no filler: the first AllGather lands too late --
                        # an eager wo(0,0) here would stall the whole PE
                        # stream behind its gathered-chunk loads
                        for _ in chunk:
                            pass
                    else:
                        import itertools
                        filler = itertools.chain(wo_gen(0, 2), wo_gen(0, 3))
                        for _ in chunk:
                            next(filler, None)
                            next(filler, None)
                        for _ in filler:
                            pass
                        for t in range(4):
                            for _ in wo_gen(1, t):
                                pass

            if debug:
                for b in range(B):
                    for h in range(2):
                        nc.sync.dma_start(
                            dbg["qrot_d"][:, h * NS + b * S: h * NS + (b + 1) * S],
                            qrot[b][:, h, :])
                    nc.sync.dma_start(dbg["krot_d"][:, b * S:(b + 1) * S], krot[b][:])
                    nc.sync.dma_start(
                        dbg["vnat_d"].rearrange("p (bb d) -> p bb d", bb=NS // P)
                        [:, b * (S // P):(b + 1) * (S // P), :], vnat[b][:])
                    for t in range(4):
                        nc.sync.dma_start(
                            dbg["ag_d"][:, b * S + t * 512: b * S + (t + 1) * 512],
                            ag_out[b][t][:])

    nc.compile()
    return nc


def _prep_inputs(x, freqs_cos, freqs_sin, wq, wk, wv, wo):
    x = np.asarray(x, np.float32).reshape(NS, DIM)
    xT = np.ascontiguousarray(
        x.T.reshape(MB, P, 8, 512).transpose(0, 2, 1, 3)).astype(BF)
    cos = np.asarray(freqs_cos, np.float32)
    sin = np.asarray(freqs_sin, np.float32)
    cosT = np.ascontiguousarray(np.tile(cos, (B, 1)).T).astype(BF)
    sinT = np.ascontiguousarray(np.tile(sin, (B, 1)).T).astype(BF)

    perm = np.r_[np.arange(0, HD, 2), np.arange(1, HD, 2)]
    scale = np.float32(1.0 / np.sqrt(HD))
    wq = np.asarray(wq, np.float32) * scale
    wk = np.asarray(wk, np.float32)
    wv = np.asarray(wv, np.float32)
    wo = np.asarray(wo, np.float32)

    # 0/1 keep-mask for the rel=3 diagonal block; rel=r reads the slice
    # shifted left by 128(3-r).  tri[p, c] = 0 iff c < 384 + p.
    cc, pp = np.meshgrid(np.arange(512), np.arange(P))
    tri = np.ascontiguousarray((cc >= pp + 384).astype(np.float32)).astype(BF)

    def tile_w(w):
        # (2048, d) -> (128, 16*d): row mi holds [mb, d] contiguously
        d = w.shape[1]
        return np.ascontiguousarray(
            w.reshape(MB, P, d).transpose(1, 0, 2).reshape(P, MB * d)).astype(BF)

    in_maps = []
    for c in range(NC):
        wq_c = wq[:, c * 256:(c + 1) * 256]
        wq_cp = np.concatenate([wq_c[:, h * HD + perm] for h in range(2)], axis=1)
        in_maps.append({
            "xT": xT,
            "wq_c": tile_w(wq_cp),
            "wk_c": tile_w(wk[:, c * HD:(c + 1) * HD][:, perm]),
            "wv_c": tile_w(wv[:, c * HD:(c + 1) * HD]),
            "wo_c": tile_w(wo[:, c * 256:(c + 1) * 256]),
            "cosT": cosT,
            "sinT": sinT,
            "tri": tri,
        })
    return in_maps


def _run(inputs, trace=False, **kw):
    from concourse.bass_utils import run_bass_kernel_spmd

    if "nc" not in _cache:
        _cache["nc"] = _build()
    nc = _cache["nc"]
    in_maps = _prep_inputs(**inputs)
    res = run_bass_kernel_spmd(
        nc, in_maps, core_ids=list(range(NC)), trace=trace, **kw
    )
    out = np.empty((NS, DIM), np.float32)
    for c in range(NC):
        out[:, c * 256:(c + 1) * 256] = res.results[c]["outT"].T
    return out.reshape(B, S, DIM), res


def kernel(**inputs) -> np.ndarray:
    out, _ = _run(inputs, trace=False)
    return out
